# revision 28
# baseline (speedup 1.0000x reference)
"""BertSelfAttention with gated prompt-prefix branch on 8 Trainium2 cores.

Sharding: data-parallel over batch (B=8 -> 1 batch element per core), no
collectives. Per core, the attention pipeline runs in a transposed
[feature, seq] layout so softmax statistics ride through the matmuls.

v2 schedule: the ScalarE (ACT) exp stream is the critical resource
(~125us of exp work that only ACT can do).  The kernel is organized as
six "pair windows" (one per head-pair) paced by the 16 exp ops of that
pair's scores.  All other PE work -- remaining Q/K projection chunks,
V/prompt projections, the previous pair's ctx matmuls and prefix branch
-- is interleaved between score matmuls as filler so the exp stream
starts ~8us into the kernel (right after chunk-0 Q/K projections) and
never waits on a phase boundary.

  qT/kT = W @ hsT          [768, 1024]  (bf16, PE), chunk c feeds pair c
  v_aug = hs @ WvT_aug     [1024, 780]  natural layout, col 65h+64 = ones
  scoresT_h = kh @ qh.T    [t, s] via K=64 row-tiled matmuls
  expT = exp(SCALE*scoresT) (e^mask folded into the V rows)
  ctxT_aug_h = v_aug_h.T @ expT_h   rows 0..63 ctx, row 64 = sum_t exp
  prefix branch identical with prompt-derived k/v; tanh(gate) folded in
  out_h = ctxT/denom + pctxT/pdenom  (reciprocal broadcast via DRAM,
                                      in-place DVE normalize + combine)

Output is produced as outT [768, 1024] fp32 per core; the host transposes
and stacks to [8, 1024, 768].
"""

import numpy as np
import ml_dtypes

import concourse.bass as bass
import concourse.mybir as mybir
import concourse.tile as tile
from concourse.bass_utils import run_bass_kernel_spmd
from concourse.vector_clock import ScopedClock


class SplitDrainTileContext(tile.TileContext):
    """This walrus build rejects >2 sync waits on the kernel-tail Drain
    ("Too many sync wait commands"); split them across SP nops instead."""

    def _drain_and_barrier(self, tick_clock, wait_clock):
        probe = self.nc.sync.nop(nofuse=True, hint="drain_wait_split")
        wait_clock.add_sem_waits(
            probe.ins, ScopedClock({None: tick_clock.global_clock})
        )
        waits = list(probe.ins.sync_info.on_wait or [])
        if len(waits) > 1:
            probe.ins.sync_info.on_wait = waits[:1]
            for i in range(1, len(waits)):
                extra = self.nc.sync.nop(nofuse=True, hint="drain_wait_split")
                extra.ins.sync_info = mybir.SyncInfo(
                    on_wait=waits[i : i + 1], on_update=[]
                )
        drain_inst = self.nc.sync.drain()
        if drain_inst.ins.sync_info is not None:
            drain_inst.ins.sync_info.on_wait = []
        self.nc.all_engine_barrier()
        assert self.sems is not None
        popped = self.nc._tile_sem_poison_stack.pop()
        assert popped is self._sem_poison
        self.nc.clear_and_free_semaphores(list(self.sems.allocated().values()))
        self.nc.all_engine_barrier()

F32 = mybir.dt.float32
BF16 = mybir.dt.bfloat16
FP8 = mybir.dt.float8e4
DR = mybir.MatmulPerfMode.DoubleRow
AF = mybir.ActivationFunctionType

H, DH, D = 12, 64, 768
S, AT, B = 1024, 64, 8
SCALE = 1.0 / np.sqrt(DH)
NC_D = D // 128  # 6 chunks over feature dim
NC_S = S // 128  # 8 chunks over sequence dim
PAIRS = H // 2  # 6 head pairs
VW = H * (DH + 1)  # 780: v with per-head ones column

_CACHE = {}
LAST_RESULTS = None


def _split_sync_waits(nc, cap=1):
    """Walrus on this image allows very few sync-wait commands per
    instruction (tensor_scalar rejects 2). Hoist excess waits onto
    same-engine nops placed immediately before the instruction."""
    for bb in nc.main_func.blocks:
        cur = list(bb.instructions)
        out = []
        for inst in cur:
            si = inst.sync_info
            waits = list(si.on_wait) if si and si.on_wait else []
            if len(waits) > cap:
                for i in range(0, len(waits) - cap):
                    bi = nc.engines[inst.engine].nop(
                        nofuse=True, hint="wait_split")
                    popped = nc.cur_bb.bb.instructions.pop()
                    assert popped is bi.ins
                    bi.ins.sync_info = mybir.SyncInfo(
                        on_wait=waits[i : i + 1], on_update=[])
                    out.append(bi.ins)
                si.on_wait = waits[len(waits) - cap:]
            out.append(inst)
        bb.instructions[:] = out


def _build_nc():
    nc = bass.Bass()
    hsT = nc.dram_tensor("hsT", [D, S], BF16, kind="ExternalInput")
    wqT = nc.dram_tensor("wqT", [128, NC_D * D], BF16, kind="ExternalInput")
    wkT = nc.dram_tensor("wkT", [128, NC_D * D], BF16, kind="ExternalInput")
    wvT = nc.dram_tensor("wvT", [D, VW], BF16, kind="ExternalInput")
    bq = nc.dram_tensor("bq", [D, 1], F32, kind="ExternalInput")
    bk = nc.dram_tensor("bk", [D, 1], F32, kind="ExternalInput")
    bvaug = nc.dram_tensor("bvaug", [128, VW], F32, kind="ExternalInput")
    promptT = nc.dram_tensor("promptT", [128, NC_D * AT], BF16,
                             kind="ExternalInput")
    mask = nc.dram_tensor("mask", [S, 1], F32, kind="ExternalInput")
    gating = nc.dram_tensor("gating", [128, VW], F32, kind="ExternalInput")
    outT = nc.dram_tensor("outT", [D, S], F32, kind="ExternalOutput")

    with SplitDrainTileContext(nc) as tc:
        _emit(nc, tc, hsT, wqT, wkT, wvT, bq, bk, bvaug, promptT, mask,
              gating, outT)
    _split_sync_waits(nc)
    return nc


def _emit(nc, tc, hsT, wqT, wkT, wvT, bq, bk, bvaug, promptT, mask, gating,
          outT):
    from contextlib import ExitStack

    with ExitStack() as ctx:
        pers = ctx.enter_context(tc.tile_pool(name="pers", bufs=1))

        # ---- persistent SBUF arrays ----
        mask_sb = pers.tile([128, NC_S], F32, tag="mask")
        emask_sb = pers.tile([128, NC_S], F32, tag="emask")
        qT_sb = pers.tile([128, NC_D * S], BF16, tag="qT")
        kT_sb = pers.tile([128, NC_D * S], BF16, tag="kT")
        v_sb = pers.tile([128, NC_S * VW], BF16, tag="v")
        pkT_sb = pers.tile([128, NC_D * AT], BF16, tag="pkT")
        pv_sb = pers.tile([128, VW], BF16, tag="pv")
        warm_sb = pers.tile([128, 128], BF16, tag="warm")
        hsT_sb = pers.tile([128, NC_D * S], BF16, tag="hsT")
        wqT_sb = pers.tile([128, NC_D * D], BF16, tag="wqT")
        wkT_sb = pers.tile([128, NC_D * D], BF16, tag="wkT")
        wvT_sb = pers.tile([128, NC_D * VW], BF16, tag="wvT")
        pT_sb = pers.tile([128, NC_D * AT], BF16, tag="pT")
        bq_sb = pers.tile([128, NC_D], F32, tag="bq")
        bk_sb = pers.tile([128, NC_D], F32, tag="bk")
        bvaug_sb = pers.tile([128, VW], F32, tag="bvaug")
        graw_sb = pers.tile([128, VW], F32, tag="graw")
        gbc_sb = pers.tile([128, VW], F32, tag="gbc")
        pvtmp_sb = pers.tile([64, VW], F32, tag="pvtmp")
        ones_sb = pers.tile([1, 64], BF16, tag="ones")

        # ---- input DMAs, chunked + ordered so chunk-0 Q/K matmuls can
        # start ~2us in (hsT kc-chunks and wq/wk c-chunks land first) ----
        hsT_r = hsT.rearrange("(k p) s -> p k s", p=128)
        hs_dst = hsT_sb[:].rearrange("p (k s) -> p k s", s=S)
        wq_r = wqT.rearrange("(k p) s -> p k s", p=128)
        wk_r = wkT.rearrange("(k p) s -> p k s", p=128)
        wq_dst = wqT_sb[:].rearrange("p (k s) -> p k s", s=D)
        wk_dst = wkT_sb[:].rearrange("p (k s) -> p k s", s=D)

        wv_r = wvT.rearrange("(k p) s -> p k s", p=128)
        wv_dst = wvT_sb[:].rearrange("p (k s) -> p k s", s=VW)

        # DMA order tracks the window-0 filler schedule: QK c0 first
        # (first matmuls ~2.5us in), then c1, then hs/wv chunks for the
        # V-projection fillers, then prompt + remaining weight chunks.
        # wq/wk/pT arrive pre-laid-out [128, c-major] from the host so
        # every chunk DMA moves 1.5KB-contiguous partition lines.
        nc.sync.dma_start(hs_dst[:, 0:1, :], hsT_r[:, 0:1, :])
        nc.sync.dma_start(wqT_sb[:, 0:D], wqT[:, 0:D])
        nc.sync.dma_start(wkT_sb[:, 0:D], wkT[:, 0:D])
        nc.sync.dma_start(bq_sb[:], bq.rearrange("(c p) 1 -> p c", p=128))
        nc.sync.dma_start(bk_sb[:], bk.rearrange("(c p) 1 -> p c", p=128))
        nc.sync.dma_start(mask_sb[:], mask.rearrange("(c p) 1 -> p c", p=128))
        nc.sync.dma_start(bvaug_sb[:], bvaug[:])
        nc.sync.dma_start(graw_sb[:], gating[:])
        nc.sync.dma_start(wqT_sb[:, D:2 * D], wqT[:, D:2 * D])
        nc.sync.dma_start(wkT_sb[:, D:2 * D], wkT[:, D:2 * D])
        for kc in range(1, NC_D):
            nc.sync.dma_start(hs_dst[:, kc:kc + 1, :], hsT_r[:, kc:kc + 1, :])
        for kc in range(NC_D):
            nc.sync.dma_start(wv_dst[:, kc:kc + 1, :], wv_r[:, kc:kc + 1, :])
        nc.sync.dma_start(pT_sb[:], promptT[:])
        nc.sync.dma_start(wkT_sb[:, 2 * D:NC_D * D], wkT[:, 2 * D:NC_D * D])
        nc.sync.dma_start(wqT_sb[:, 2 * D:NC_D * D], wqT[:, 2 * D:NC_D * D])

        # e^mask folded into the V rows (masked denominator comes free)
        nc.scalar.activation(emask_sb[:], mask_sb[:], AF.Exp)
        nc.vector.memset(ones_sb[:], 1.0)

        def gating_setup():
            # tanh of the gating factors; emitted mid-window-0 so it
            # never head-of-line blocks the exp stream on its input DMA
            nc.scalar.activation(gbc_sb[:], graw_sb[:], AF.Tanh)
            ones_slots = gbc_sb[:, :].rearrange(
                "p (h e) -> p h e", h=H)[:, :, DH:DH + 1]
            nc.vector.memset(ones_slots, 1.0)

        # ---- pools ----
        exp_pool = ctx.enter_context(tc.tile_pool(name="expp", bufs=4))
        pexp_pool = ctx.enter_context(tc.tile_pool(name="pexpp", bufs=3))
        st_pool = ctx.enter_context(
            tc.tile_pool(name="stp", bufs=2, space="PSUM"))
        ctx_pool = ctx.enter_context(
            tc.tile_pool(name="ctxp", bufs=2, space="PSUM"))
        norm_pool = ctx.enter_context(tc.tile_pool(name="normp", bufs=2))
        dscr_pool = ctx.enter_context(
            tc.tile_pool(name="dscr", bufs=2, space="DRAM"))

        # ================= work-unit generators =================
        def qk_half(c, which):
            """Project one of Q/K for feature chunk c -> qT/kT chunk c."""
            w_sb, b_sb, o_sb = ((wqT_sb, bq_sb, qT_sb) if which == 0
                                else (wkT_sb, bk_sb, kT_sb))
            ps = st_pool.tile([128, S], F32, tag="st",
                              name=f"qk_{c}_{which}")
            for kc in range(NC_D):
                lhsT = w_sb[:, c * D + kc * 128: c * D + (kc + 1) * 128]
                for sb in range(2):
                    nc.tensor.matmul(
                        ps[:, sb * 512:(sb + 1) * 512], lhsT,
                        hsT_sb[:, kc * S + sb * 512: kc * S + (sb + 1) * 512],
                        start=(kc == 0), stop=(kc == NC_D - 1))
            nc.vector.tensor_scalar_add(o_sb[:, c * S:(c + 1) * S],
                                        ps[:], b_sb[:, c:c + 1])

        def v_chunk(sc):
            """V projection s-chunk sc (natural layout, ones column)."""
            ps = st_pool.tile([128, S], F32, tag="st", name=f"v_{sc}")
            for kc in range(NC_D):
                lhsT = hsT_sb[:, kc * S + sc * 128: kc * S + (sc + 1) * 128]
                nc.tensor.matmul(ps[:, 0:512], lhsT,
                                 wvT_sb[:, kc * VW: kc * VW + 512],
                                 start=(kc == 0), stop=(kc == NC_D - 1))
                nc.tensor.matmul(ps[:, 512:VW], lhsT,
                                 wvT_sb[:, kc * VW + 512: (kc + 1) * VW],
                                 start=(kc == 0), stop=(kc == NC_D - 1))
            vt = norm_pool.tile([128, VW], F32, tag="vtmp", bufs=2,
                                name=f"vt{sc}")
            nc.vector.tensor_add(vt[:], ps[:, 0:VW], bvaug_sb[:])
            nc.vector.tensor_scalar_mul(v_sb[:, sc * VW:(sc + 1) * VW],
                                        vt[:], emask_sb[:, sc:sc + 1])

        def prompt_k(c):
            ps = st_pool.tile([128, S], F32, tag="st", name=f"pk_{c}")
            for kc in range(NC_D):
                nc.tensor.matmul(
                    ps[:, 0:AT],
                    wkT_sb[:, c * D + kc * 128: c * D + (kc + 1) * 128],
                    pT_sb[:, kc * AT:(kc + 1) * AT],
                    start=(kc == 0), stop=(kc == NC_D - 1))
            nc.vector.tensor_scalar_add(pkT_sb[:, c * AT:(c + 1) * AT],
                                        ps[:, 0:AT], bk_sb[:, c:c + 1])

        def prompt_v():
            ps = st_pool.tile([128, S], F32, tag="st", name="pvps")
            for kc in range(NC_D):
                lhsT = pT_sb[:, kc * AT:(kc + 1) * AT]
                nc.tensor.matmul(ps[0:AT, 0:512], lhsT,
                                 wvT_sb[:, kc * VW: kc * VW + 512],
                                 start=(kc == 0), stop=(kc == NC_D - 1))
                nc.tensor.matmul(ps[0:AT, 512:VW], lhsT,
                                 wvT_sb[:, kc * VW + 512: (kc + 1) * VW],
                                 start=(kc == 0), stop=(kc == NC_D - 1))
            nc.vector.tensor_add(pvtmp_sb[:], ps[0:AT, 0:VW],
                                 bvaug_sb[0:AT, :])
            nc.vector.tensor_mul(pv_sb[0:AT, :], pvtmp_sb[:],
                                 gbc_sb[0:AT, :])
            nc.sync.dma_start(pv_sb[AT:128, :], pv_sb[0:AT, :])

        def score_unit(c, tci, half, exp_ab, keep_warm=False):
            hp = half * 64
            st = st_pool.tile([128, S], F32, tag="st",
                              name=f"st_{c}_{tci}_{half}")
            if keep_warm:
                # full-array dummy matmul: keeps the HAM activity monitor
                # fed so the PE clock stays at 2.4GHz in windows that have
                # no full-width projection fillers left.  The scores
                # matmul below starts a fresh accumulation group, so the
                # garbage it writes is overwritten.
                nc.tensor.matmul(st[:, 0:128], warm_sb[:],
                                 warm_sb[:, 0:128])
            lhsT = kT_sb[hp:hp + 64,
                         c * S + tci * 128: c * S + (tci + 1) * 128]
            for sb in range(2):
                nc.tensor.matmul(
                    st[:, sb * 512:(sb + 1) * 512], lhsT,
                    qT_sb[hp:hp + 64,
                          c * S + sb * 512: c * S + (sb + 1) * 512],
                    tile_position=(hp, 0))
            nc.scalar.activation(exp_ab[half][:, tci * S:(tci + 1) * S],
                                 st[:], AF.Exp, scale=SCALE)

        def ctx_unit(cps, half, tci, c, exp_ab):
            h = 2 * c + half
            lhsT = v_sb[:, tci * VW + h * 65: tci * VW + h * 65 + 65]
            for sb in range(2):
                nc.tensor.matmul(
                    cps[:, sb * 512:(sb + 1) * 512], lhsT,
                    exp_ab[half][:, tci * S + sb * 512:
                                 tci * S + (sb + 1) * 512],
                    start=(tci == 0), stop=(tci == NC_S - 1))

        def prefix_scores(c, pexp):
            psp = st_pool.tile([128, S], F32, tag="st", name=f"psp_{c}")
            for half in range(2):
                hp = half * 64
                for sb in range(2):
                    nc.tensor.matmul(
                        psp[hp:hp + 64, sb * 512:(sb + 1) * 512],
                        pkT_sb[hp:hp + 64, c * AT:(c + 1) * AT],
                        qT_sb[hp:hp + 64,
                              c * S + sb * 512: c * S + (sb + 1) * 512],
                        tile_position=(hp, hp))
            nc.scalar.activation(pexp[:], psp[:], AF.Exp, scale=SCALE)

        def prefix_ctx(c, half, pexp):
            h = 2 * c + half
            hp = half * 64
            pps = ctx_pool.tile([65, S], F32, tag="ctx",
                                name=f"pps_{c}_{half}")
            for sb in range(2):
                nc.tensor.matmul(
                    pps[:, sb * 512:(sb + 1) * 512],
                    pv_sb[hp:hp + 64, h * 65: h * 65 + 65],
                    pexp[hp:hp + 64, sb * 512:(sb + 1) * 512],
                    tile_position=(hp, 0))
            return pps

        def finish_stage1(c, half, cps, pps):
            """Evacuate ctx+prefix numerators/denominators to SBUF, then
            launch the reciprocal-broadcast round trip (no PE involved)."""
            h = 2 * c + half
            nt = norm_pool.tile([65, 2 * S], F32, tag="nt", bufs=3,
                                name=f"nt_{c}_{half}")
            nc.vector.tensor_copy(nt[:, 0:S], cps[:])
            nc.vector.tensor_copy(nt[:, S:2 * S], pps[:])
            dresh = norm_pool.tile([128, 16], F32, tag="dresh", bufs=2,
                                   name=f"dr_{c}_{half}")
            nc.sync.dma_start(dresh[:, 0:8], nt[64:65, 0:S])
            nc.sync.dma_start(dresh[:, 8:16], nt[64:65, S:2 * S])
            rrec = norm_pool.tile([128, 16], BF16, tag="rrec", bufs=2,
                                  name=f"rr_{c}_{half}")
            with nc.allow_low_precision(reason="bf16 softmax recip bcast"):
                nc.vector.reciprocal(rrec[:], dresh[:])
            r_d = dscr_pool.tile([1, 2 * S], BF16, tag="rd", bufs=2,
                                 name=f"rd_{c}_{half}")
            nc.sync.dma_start(r_d[0:1, 0:S], rrec[:, 0:8])
            nc.sync.dma_start(r_d[0:1, S:2 * S], rrec[:, 8:16])
            r_bc = norm_pool.tile([64, 2 * S], BF16, tag="rbc", bufs=2,
                                  name=f"rbc_{c}_{half}")
            r_src = bass.AP(r_d[:].tensor, r_d[:].offset,
                            [[0, 64], [1, 2 * S]])
            nc.sync.dma_start(r_bc[:], r_src)
            return (h, nt, r_bc)

        def finish_stage2(state):
            """Normalize in place, combine, write out (all DVE)."""
            h, nt, r_bc = state
            nc.vector.tensor_mul(nt[0:64, 0:S], nt[0:64, 0:S], r_bc[:, 0:S])
            nc.vector.tensor_mul(nt[0:64, S:2 * S], nt[0:64, S:2 * S],
                                 r_bc[:, S:2 * S])
            nc.vector.tensor_add(nt[0:64, 0:S], nt[0:64, 0:S],
                                 nt[0:64, S:2 * S])
            nc.sync.dma_start(outT[h * 64:(h + 1) * 64, :], nt[0:64, 0:S])

        # ================= the window schedule =================
        # HAM warm-up: ~4us of tiny full-array matmuls on scratch data
        # while the input DMAs stream, so the PE clock is at 2.4GHz when
        # the first projection matmul issues.
        nc.vector.memset(warm_sb[:], 0.0)
        wps = st_pool.tile([128, S], F32, tag="st", name="warmps")
        for i in range(50):
            nc.tensor.matmul(wps[:, 0:128], warm_sb[:], warm_sb[:, 0:128])

        # chunk-0 Q/K projections up front (pair 0 inputs); everything
        # else rides as window fillers behind the exp stream
        qk_half(0, 0)
        qk_half(0, 1)

        # filler units per window (emitted between score units).
        # window 0 also carries the prompt-K projections so pair-0's
        # prefix branch can run at the uniform half-lag schedule.
        fillers = [[] for _ in range(PAIRS)]
        fillers[0] = [lambda: qk_half(1, 0), lambda: qk_half(1, 1)] + [
            lambda sc=sc: v_chunk(sc) for sc in range(5)] + [
            lambda cc=cc: prompt_k(cc) for cc in range(NC_D)]
        fillers[1] = [prompt_v, lambda: qk_half(2, 0), lambda: qk_half(2, 1),
                      lambda: qk_half(3, 0), lambda: qk_half(3, 1)]
        fillers[2] = [lambda: qk_half(4, 0), lambda: qk_half(4, 1)]
        fillers[3] = [lambda: qk_half(5, 0), lambda: qk_half(5, 1)]
        # v s5-7 must land before pair-0's leftover ctx units read them:
        # v5/v6 ahead of the tci-0 leftovers, v7 ahead of the tci-1 ones
        front = [{} for _ in range(PAIRS)]
        front[1] = {0: [lambda: v_chunk(5)], 1: [lambda: v_chunk(6)],
                    2: [lambda: v_chunk(7)]}

        # Half-lag schedule per window c:
        #   tci 0-1: leftover ctx units (ti 5,6,7) of pair c-1
        #   tci 2:   prefix-ctx + finish stage1 of pair c-1
        #   tci 3-7: ctx units ti 0..4 of pair c (1 ti per tci)
        #   tci 6:   finish stage2 of pair c-1 (r_bc round trip done)
        #   end:     prefix scores of pair c (joins the exp stream)
        prev = None      # (c, exp_ab, pexp, cps_ab, leftover list)
        stage2_q = []

        for c in range(PAIRS):
            exp_ab = [exp_pool.tile([128, NC_S * S], BF16, tag="exp",
                                    name=f"exp_{c}_{i}") for i in range(2)]
            fl = list(fillers[c])
            nfill = len(fl)
            cps_ab = None
            states = []
            # window 5 runs its whole ctx in-window (no next window)
            ctx_plan = [(half, ti) for ti in range(NC_S)
                        for half in range(2)]
            quota = 16 if c == PAIRS - 1 else 10
            for tci in range(NC_S):
                for half in range(2):
                    score_unit(c, tci, half, exp_ab)
                for fn in front[c].get(tci, ()):
                    fn()
                if c == 0 and tci == 1:
                    gating_setup()
                if prev is not None:
                    pc, pexp_ab, ppexp, pcps, pleft = prev
                    if tci < 3:
                        for _ in range(2):
                            if pleft:
                                half, ti = pleft.pop(0)
                                ctx_unit(pcps[half], half, ti, pc, pexp_ab)
                    if tci == 3:
                        for half in range(2):
                            pps = prefix_ctx(pc, half, ppexp)
                            states.append((half, pps))
                        for half, pps in states:
                            stage2_q.append(
                                finish_stage1(pc, half, pcps[half], pps))
                if tci >= 3:
                    if cps_ab is None:
                        cps_ab = {hh: ctx_pool.tile(
                            [65, S], F32, tag="ctx", name=f"cps_{c}_{hh}")
                            for hh in range(2)}
                    done = 16 - len(ctx_plan)
                    want = quota * (tci - 2) // 5 - done
                    for _ in range(max(0, want)):
                        if ctx_plan:
                            half, ti = ctx_plan.pop(0)
                            ctx_unit(cps_ab[half], half, ti, c, exp_ab)
                if tci == 6:
                    for stt in stage2_q:
                        finish_stage2(stt)
                    stage2_q = []
                while fl and len(fl) > nfill * (7 - tci) // 8:
                    fl.pop(0)()

            pexp = pexp_pool.tile([128, S], BF16, tag="pexp",
                                  name=f"pexp_{c}")
            prefix_scores(c, pexp)
            prev = (c, exp_ab, pexp, cps_ab, ctx_plan)

        # ---- tail: pair 5 remaining ctx + stage-parallel finish ----
        pc, pexp_ab, ppexp, pcps, pleft = prev
        for half, ti in pleft:
            ctx_unit(pcps[half], half, ti, pc, pexp_ab)
        for stt in stage2_q:
            finish_stage2(stt)
        pps_l = [prefix_ctx(pc, half, ppexp) for half in range(2)]
        nts, rrecs, rps = [], [], []
        for half in range(2):
            nt = norm_pool.tile([65, 2 * S], F32, tag="nt", bufs=3,
                                name=f"ntt_{half}")
            nc.vector.tensor_copy(nt[:, 0:S], pcps[half][:])
            nc.vector.tensor_copy(nt[:, S:2 * S], pps_l[half][:])
            nts.append(nt)
        for half in range(2):
            dresh = norm_pool.tile([128, 16], F32, tag="dresh", bufs=2,
                                   name=f"drt_{half}")
            nc.sync.dma_start(dresh[:, 0:8], nts[half][64:65, 0:S])
            nc.sync.dma_start(dresh[:, 8:16], nts[half][64:65, S:2 * S])
            rrec = norm_pool.tile([128, 16], BF16, tag="rrec", bufs=2,
                                  name=f"rrt_{half}")
            with nc.allow_low_precision(reason="softmax recip bcast"):
                nc.vector.reciprocal(rrec[:], dresh[:])
            rrecs.append(rrec)
        for half in range(2):
            r_row = norm_pool.tile([1, 2 * S], BF16, tag="rrow", bufs=2,
                                   name=f"rwt_{half}")
            nc.sync.dma_start(r_row[0:1, 0:S], rrecs[half][:, 0:8])
            nc.sync.dma_start(r_row[0:1, S:2 * S], rrecs[half][:, 8:16])
            rps_c = st_pool.tile([128, S], F32, tag="st",
                                 name=f"rpc_{half}")
            rps_p = ctx_pool.tile([65, S], F32, tag="ctx",
                                  name=f"rpp_{half}")
            for sb in range(2):
                nc.tensor.matmul(rps_c[0:64, sb * 512:(sb + 1) * 512],
                                 ones_sb[:],
                                 r_row[0:1, sb * 512:(sb + 1) * 512])
                nc.tensor.matmul(rps_p[0:64, sb * 512:(sb + 1) * 512],
                                 ones_sb[:],
                                 r_row[0:1, S + sb * 512: S + (sb + 1) * 512])
            rps.append((rps_c, rps_p))
        for half in range(2):
            h = 2 * pc + half
            nt = nts[half]
            rps_c, rps_p = rps[half]
            nc.vector.tensor_mul(nt[0:64, 0:S], nt[0:64, 0:S],
                                 rps_c[0:64, :])
            nc.vector.tensor_mul(nt[0:64, S:2 * S], nt[0:64, S:2 * S],
                                 rps_p[0:64, :])
            nc.gpsimd.tensor_add(nt[0:64, 0:S], nt[0:64, 0:S],
                                 nt[0:64, S:2 * S])
            nc.sync.dma_start(outT[h * 64:(h + 1) * 64, :], nt[0:64, 0:S])


def _prep_inputs(hidden_states, prompt_tokens, gating_factor, attention_mask,
                 Wq, bq, Wk, bk, Wv, bv):
    bf = ml_dtypes.bfloat16
    hs = np.asarray(hidden_states, np.float32)
    mask = np.asarray(attention_mask, np.float32).reshape(B, S)
    def _cmaj(w):
        # [din, dout] -> [128, (c, kc, j)] so each chunk DMA is a plain
        # contiguous column-slice (1.5KB per partition line)
        wt = np.asarray(w, np.float32).T.reshape(NC_D, 128, NC_D, 128)
        return np.ascontiguousarray(
            wt.transpose(1, 2, 0, 3).reshape(128, NC_D * D)).astype(bf)

    wqT = _cmaj(Wq)
    wkT = _cmaj(Wk)
    # augmented WvT: [din, 780], col 65h+j = Wv.T[:, 64h+j], col 65h+64 = 0
    wvT_f = np.asarray(Wv, np.float32).T  # [din, dout]
    wvT_aug = np.zeros((D, VW), np.float32)
    idx = np.arange(D)
    aug_cols = (idx // DH) * (DH + 1) + (idx % DH)
    wvT_aug[:, aug_cols] = wvT_f
    wvT_aug = wvT_aug.astype(bf)
    bq_c = np.asarray(bq, np.float32).reshape(D, 1)
    bk_c = np.asarray(bk, np.float32).reshape(D, 1)
    bv_aug = np.zeros(VW, np.float32)
    bv_aug[aug_cols] = np.asarray(bv, np.float32)
    bv_aug[DH::DH + 1] = 1.0
    bvaug_bc = np.ascontiguousarray(
        np.broadcast_to(bv_aug, (128, VW)), np.float32)
    p0T = np.asarray(prompt_tokens, np.float32)[0].T.reshape(NC_D, 128, AT)
    pT = np.ascontiguousarray(
        p0T.transpose(1, 0, 2).reshape(128, NC_D * AT)).astype(bf)
    gat_row = np.repeat(
        np.asarray(gating_factor, np.float32).reshape(H), DH + 1)
    gat = np.ascontiguousarray(
        np.broadcast_to(gat_row, (128, VW)), np.float32)

    shared = dict(wqT=wqT, wkT=wkT, wvT=wvT_aug, bq=bq_c, bk=bk_c,
                  bvaug=bvaug_bc, promptT=pT, gating=gat)
    in_maps = []
    for b in range(B):
        m = dict(shared)
        m["hsT"] = np.ascontiguousarray(hs[b].T).astype(bf)
        m["mask"] = np.ascontiguousarray(mask[b].reshape(S, 1))
        in_maps.append(m)
    return in_maps


def kernel(**inputs):
    global LAST_RESULTS
    if "nc" not in _CACHE:
        _CACHE["nc"] = _build_nc()
    nc = _CACHE["nc"]
    in_maps = _prep_inputs(**inputs)
    res = None
    for attempt in range(3):
        try:
            res = run_bass_kernel_spmd(nc, in_maps, list(range(B)))
            break
        except ModuleNotFoundError:
            # BASS_TRACE set but this image lacks antenv.axon_hooks
            import os

            os.environ["BASS_NEVER_TRACE"] = "1"
            if attempt == 2:
                raise
        except Exception:
            # transient NRT_EXEC_UNIT_UNRECOVERABLE on a cold device has
            # been observed; a retry on the same session recovers
            if attempt == 2:
                raise
    LAST_RESULTS = res
    out = np.empty((B, S, D), np.float32)
    for b in range(B):
        out[b] = res.results[b]["outT"].T
    return out


# revision 29
# speedup vs baseline: 1.0476x; 1.0476x over previous
"""BertSelfAttention with gated prompt-prefix branch on 8 Trainium2 cores.

Sharding: data-parallel over batch (B=8 -> 1 batch element per core), no
collectives. Per core, the attention pipeline runs in a transposed
[feature, seq] layout so softmax statistics ride through the matmuls.

v2 schedule: the ScalarE (ACT) exp stream is the critical resource
(~125us of exp work that only ACT can do).  The kernel is organized as
six "pair windows" (one per head-pair) paced by the 16 exp ops of that
pair's scores.  All other PE work -- remaining Q/K projection chunks,
V/prompt projections, the previous pair's ctx matmuls and prefix branch
-- is interleaved between score matmuls as filler so the exp stream
starts ~8us into the kernel (right after chunk-0 Q/K projections) and
never waits on a phase boundary.

  qT/kT = W @ hsT          [768, 1024]  (bf16, PE), chunk c feeds pair c
  v_aug = hs @ WvT_aug     [1024, 780]  natural layout, col 65h+64 = ones
  scoresT_h = kh @ qh.T    [t, s] via K=64 row-tiled matmuls
  expT = exp(SCALE*scoresT) (e^mask folded into the V rows)
  ctxT_aug_h = v_aug_h.T @ expT_h   rows 0..63 ctx, row 64 = sum_t exp
  prefix branch identical with prompt-derived k/v; tanh(gate) folded in
  out_h = ctxT/denom + pctxT/pdenom  (reciprocal broadcast via DRAM,
                                      in-place DVE normalize + combine)

Output is produced as outT [768, 1024] fp32 per core; the host transposes
and stacks to [8, 1024, 768].
"""

import numpy as np
import ml_dtypes

import concourse.bass as bass
import concourse.mybir as mybir
import concourse.tile as tile
from concourse.bass_utils import run_bass_kernel_spmd
from concourse.vector_clock import ScopedClock


class SplitDrainTileContext(tile.TileContext):
    """This walrus build rejects >2 sync waits on the kernel-tail Drain
    ("Too many sync wait commands"); split them across SP nops instead."""

    def _drain_and_barrier(self, tick_clock, wait_clock):
        probe = self.nc.sync.nop(nofuse=True, hint="drain_wait_split")
        wait_clock.add_sem_waits(
            probe.ins, ScopedClock({None: tick_clock.global_clock})
        )
        waits = list(probe.ins.sync_info.on_wait or [])
        if len(waits) > 1:
            probe.ins.sync_info.on_wait = waits[:1]
            for i in range(1, len(waits)):
                extra = self.nc.sync.nop(nofuse=True, hint="drain_wait_split")
                extra.ins.sync_info = mybir.SyncInfo(
                    on_wait=waits[i : i + 1], on_update=[]
                )
        drain_inst = self.nc.sync.drain()
        if drain_inst.ins.sync_info is not None:
            drain_inst.ins.sync_info.on_wait = []
        self.nc.all_engine_barrier()
        assert self.sems is not None
        popped = self.nc._tile_sem_poison_stack.pop()
        assert popped is self._sem_poison
        self.nc.clear_and_free_semaphores(list(self.sems.allocated().values()))
        self.nc.all_engine_barrier()

F32 = mybir.dt.float32
BF16 = mybir.dt.bfloat16
FP8 = mybir.dt.float8e4
DR = mybir.MatmulPerfMode.DoubleRow
AF = mybir.ActivationFunctionType

H, DH, D = 12, 64, 768
S, AT, B = 1024, 64, 8
SCALE = 1.0 / np.sqrt(DH)
NC_D = D // 128  # 6 chunks over feature dim
NC_S = S // 128  # 8 chunks over sequence dim
PAIRS = H // 2  # 6 head pairs
VW = H * (DH + 1)  # 780: v with per-head ones column

_CACHE = {}
LAST_RESULTS = None


def _split_sync_waits(nc, cap=1):
    """Walrus on this image allows very few sync-wait commands per
    instruction (tensor_scalar rejects 2). Hoist excess waits onto
    same-engine nops placed immediately before the instruction."""
    for bb in nc.main_func.blocks:
        cur = list(bb.instructions)
        out = []
        for inst in cur:
            si = inst.sync_info
            waits = list(si.on_wait) if si and si.on_wait else []
            if len(waits) > cap:
                for i in range(0, len(waits) - cap):
                    bi = nc.engines[inst.engine].nop(
                        nofuse=True, hint="wait_split")
                    popped = nc.cur_bb.bb.instructions.pop()
                    assert popped is bi.ins
                    bi.ins.sync_info = mybir.SyncInfo(
                        on_wait=waits[i : i + 1], on_update=[])
                    out.append(bi.ins)
                si.on_wait = waits[len(waits) - cap:]
            out.append(inst)
        bb.instructions[:] = out


def _build_nc():
    nc = bass.Bass()
    hsT = nc.dram_tensor("hsT", [D, S], BF16, kind="ExternalInput")
    wqT = nc.dram_tensor("wqT", [128, NC_D * D], BF16, kind="ExternalInput")
    wkT = nc.dram_tensor("wkT", [128, NC_D * D], BF16, kind="ExternalInput")
    wvT = nc.dram_tensor("wvT", [D, VW], BF16, kind="ExternalInput")
    bq = nc.dram_tensor("bq", [D, 1], F32, kind="ExternalInput")
    bk = nc.dram_tensor("bk", [D, 1], F32, kind="ExternalInput")
    bvaug = nc.dram_tensor("bvaug", [128, VW], F32, kind="ExternalInput")
    promptT = nc.dram_tensor("promptT", [128, NC_D * AT], BF16,
                             kind="ExternalInput")
    mask = nc.dram_tensor("mask", [S, 1], F32, kind="ExternalInput")
    gating = nc.dram_tensor("gating", [128, VW], F32, kind="ExternalInput")
    outT = nc.dram_tensor("outT", [D, S], F32, kind="ExternalOutput")

    with SplitDrainTileContext(nc) as tc:
        _emit(nc, tc, hsT, wqT, wkT, wvT, bq, bk, bvaug, promptT, mask,
              gating, outT)
    _split_sync_waits(nc)
    return nc


def _emit(nc, tc, hsT, wqT, wkT, wvT, bq, bk, bvaug, promptT, mask, gating,
          outT):
    from contextlib import ExitStack

    with ExitStack() as ctx:
        pers = ctx.enter_context(tc.tile_pool(name="pers", bufs=1))

        # ---- persistent SBUF arrays ----
        mask_sb = pers.tile([128, NC_S], F32, tag="mask")
        emask_sb = pers.tile([128, NC_S], F32, tag="emask")
        qT_sb = pers.tile([128, NC_D * S], BF16, tag="qT")
        kT_sb = pers.tile([128, NC_D * S], BF16, tag="kT")
        v_sb = pers.tile([128, NC_S * VW], BF16, tag="v")
        pkT_sb = pers.tile([128, NC_D * AT], BF16, tag="pkT")
        pv_sb = pers.tile([128, VW], BF16, tag="pv")
        warm_sb = pers.tile([128, 128], BF16, tag="warm")
        hsT_sb = pers.tile([128, NC_D * S], BF16, tag="hsT")
        wqT_sb = pers.tile([128, NC_D * D], BF16, tag="wqT")
        wkT_sb = pers.tile([128, NC_D * D], BF16, tag="wkT")
        wvT_sb = pers.tile([128, NC_D * VW], BF16, tag="wvT")
        pT_sb = pers.tile([128, NC_D * AT], BF16, tag="pT")
        bq_sb = pers.tile([128, NC_D], F32, tag="bq")
        bk_sb = pers.tile([128, NC_D], F32, tag="bk")
        bvaug_sb = pers.tile([128, VW], F32, tag="bvaug")
        graw_sb = pers.tile([128, VW], F32, tag="graw")
        gbc_sb = pers.tile([128, VW], F32, tag="gbc")
        pvtmp_sb = pers.tile([64, VW], F32, tag="pvtmp")
        ones_sb = pers.tile([1, 64], BF16, tag="ones")

        # ---- input DMAs, chunked + ordered so chunk-0 Q/K matmuls can
        # start ~2us in (hsT kc-chunks and wq/wk c-chunks land first) ----
        hsT_r = hsT.rearrange("(k p) s -> p k s", p=128)
        hs_dst = hsT_sb[:].rearrange("p (k s) -> p k s", s=S)
        wq_r = wqT.rearrange("(k p) s -> p k s", p=128)
        wk_r = wkT.rearrange("(k p) s -> p k s", p=128)
        wq_dst = wqT_sb[:].rearrange("p (k s) -> p k s", s=D)
        wk_dst = wkT_sb[:].rearrange("p (k s) -> p k s", s=D)

        wv_r = wvT.rearrange("(k p) s -> p k s", p=128)
        wv_dst = wvT_sb[:].rearrange("p (k s) -> p k s", s=VW)

        # DMA order tracks the window-0 filler schedule: QK c0 first
        # (first matmuls ~2.5us in), then c1, then hs/wv chunks for the
        # V-projection fillers, then prompt + remaining weight chunks.
        # wq/wk/pT arrive pre-laid-out [128, c-major] from the host so
        # every chunk DMA moves 1.5KB-contiguous partition lines.
        nc.sync.dma_start(hs_dst[:, 0:1, :], hsT_r[:, 0:1, :])
        nc.sync.dma_start(wqT_sb[:, 0:D], wqT[:, 0:D])
        nc.sync.dma_start(wkT_sb[:, 0:D], wkT[:, 0:D])
        nc.sync.dma_start(bq_sb[:], bq.rearrange("(c p) 1 -> p c", p=128))
        nc.sync.dma_start(bk_sb[:], bk.rearrange("(c p) 1 -> p c", p=128))
        nc.sync.dma_start(mask_sb[:], mask.rearrange("(c p) 1 -> p c", p=128))
        nc.sync.dma_start(bvaug_sb[:], bvaug[:])
        nc.sync.dma_start(graw_sb[:], gating[:])
        nc.sync.dma_start(wqT_sb[:, D:2 * D], wqT[:, D:2 * D])
        nc.sync.dma_start(wkT_sb[:, D:2 * D], wkT[:, D:2 * D])
        for kc in range(1, NC_D):
            nc.sync.dma_start(hs_dst[:, kc:kc + 1, :], hsT_r[:, kc:kc + 1, :])
        for kc in range(NC_D):
            nc.sync.dma_start(wv_dst[:, kc:kc + 1, :], wv_r[:, kc:kc + 1, :])
        nc.sync.dma_start(pT_sb[:], promptT[:])
        nc.sync.dma_start(wkT_sb[:, 2 * D:NC_D * D], wkT[:, 2 * D:NC_D * D])
        nc.sync.dma_start(wqT_sb[:, 2 * D:NC_D * D], wqT[:, 2 * D:NC_D * D])

        # e^mask folded into the V rows (masked denominator comes free)
        nc.scalar.activation(emask_sb[:], mask_sb[:], AF.Exp)
        nc.vector.memset(ones_sb[:], 1.0)

        def gating_setup():
            # tanh of the gating factors; emitted mid-window-0 so it
            # never head-of-line blocks the exp stream on its input DMA
            nc.scalar.activation(gbc_sb[:], graw_sb[:], AF.Tanh)
            ones_slots = gbc_sb[:, :].rearrange(
                "p (h e) -> p h e", h=H)[:, :, DH:DH + 1]
            nc.vector.memset(ones_slots, 1.0)

        # ---- pools ----
        exp_pool = ctx.enter_context(tc.tile_pool(name="expp", bufs=4))
        pexp_pool = ctx.enter_context(tc.tile_pool(name="pexpp", bufs=3))
        st_pool = ctx.enter_context(
            tc.tile_pool(name="stp", bufs=2, space="PSUM"))
        ctx_pool = ctx.enter_context(
            tc.tile_pool(name="ctxp", bufs=2, space="PSUM"))
        norm_pool = ctx.enter_context(tc.tile_pool(name="normp", bufs=2))
        dscr_pool = ctx.enter_context(
            tc.tile_pool(name="dscr", bufs=2, space="DRAM"))

        # ================= work-unit generators =================
        def qk_half(c, which):
            """Project one of Q/K for feature chunk c -> qT/kT chunk c."""
            w_sb, b_sb, o_sb = ((wqT_sb, bq_sb, qT_sb) if which == 0
                                else (wkT_sb, bk_sb, kT_sb))
            ps = st_pool.tile([128, S], F32, tag="st",
                              name=f"qk_{c}_{which}")
            for kc in range(NC_D):
                lhsT = w_sb[:, c * D + kc * 128: c * D + (kc + 1) * 128]
                for sb in range(2):
                    nc.tensor.matmul(
                        ps[:, sb * 512:(sb + 1) * 512], lhsT,
                        hsT_sb[:, kc * S + sb * 512: kc * S + (sb + 1) * 512],
                        start=(kc == 0), stop=(kc == NC_D - 1))
            nc.vector.tensor_scalar_add(o_sb[:, c * S:(c + 1) * S],
                                        ps[:], b_sb[:, c:c + 1])

        def v_chunk(sc):
            """V projection s-chunk sc (natural layout, ones column)."""
            ps = st_pool.tile([128, S], F32, tag="st", name=f"v_{sc}")
            for kc in range(NC_D):
                lhsT = hsT_sb[:, kc * S + sc * 128: kc * S + (sc + 1) * 128]
                nc.tensor.matmul(ps[:, 0:512], lhsT,
                                 wvT_sb[:, kc * VW: kc * VW + 512],
                                 start=(kc == 0), stop=(kc == NC_D - 1))
                nc.tensor.matmul(ps[:, 512:VW], lhsT,
                                 wvT_sb[:, kc * VW + 512: (kc + 1) * VW],
                                 start=(kc == 0), stop=(kc == NC_D - 1))
            vt = norm_pool.tile([128, VW], F32, tag="vtmp", bufs=2,
                                name=f"vt{sc}")
            nc.vector.tensor_add(vt[:], ps[:, 0:VW], bvaug_sb[:])
            nc.vector.tensor_scalar_mul(v_sb[:, sc * VW:(sc + 1) * VW],
                                        vt[:], emask_sb[:, sc:sc + 1])

        def prompt_k(c):
            ps = st_pool.tile([128, S], F32, tag="st", name=f"pk_{c}")
            for kc in range(NC_D):
                nc.tensor.matmul(
                    ps[:, 0:AT],
                    wkT_sb[:, c * D + kc * 128: c * D + (kc + 1) * 128],
                    pT_sb[:, kc * AT:(kc + 1) * AT],
                    start=(kc == 0), stop=(kc == NC_D - 1))
            nc.vector.tensor_scalar_add(pkT_sb[:, c * AT:(c + 1) * AT],
                                        ps[:, 0:AT], bk_sb[:, c:c + 1])

        def prompt_v():
            ps = st_pool.tile([128, S], F32, tag="st", name="pvps")
            for kc in range(NC_D):
                lhsT = pT_sb[:, kc * AT:(kc + 1) * AT]
                nc.tensor.matmul(ps[0:AT, 0:512], lhsT,
                                 wvT_sb[:, kc * VW: kc * VW + 512],
                                 start=(kc == 0), stop=(kc == NC_D - 1))
                nc.tensor.matmul(ps[0:AT, 512:VW], lhsT,
                                 wvT_sb[:, kc * VW + 512: (kc + 1) * VW],
                                 start=(kc == 0), stop=(kc == NC_D - 1))
            nc.vector.tensor_add(pvtmp_sb[:], ps[0:AT, 0:VW],
                                 bvaug_sb[0:AT, :])
            nc.vector.tensor_mul(pv_sb[0:AT, :], pvtmp_sb[:],
                                 gbc_sb[0:AT, :])
            nc.sync.dma_start(pv_sb[AT:128, :], pv_sb[0:AT, :])

        def score_unit(c, tci, half, exp_ab, keep_warm=False):
            hp = half * 64
            st = st_pool.tile([128, S], F32, tag="st",
                              name=f"st_{c}_{tci}_{half}")
            if keep_warm:
                # full-array dummy matmul: keeps the HAM activity monitor
                # fed so the PE clock stays at 2.4GHz in windows that have
                # no full-width projection fillers left.  The scores
                # matmul below starts a fresh accumulation group, so the
                # garbage it writes is overwritten.
                nc.tensor.matmul(st[:, 0:128], warm_sb[:],
                                 warm_sb[:, 0:128])
            lhsT = kT_sb[hp:hp + 64,
                         c * S + tci * 128: c * S + (tci + 1) * 128]
            for sb in range(2):
                nc.tensor.matmul(
                    st[:, sb * 512:(sb + 1) * 512], lhsT,
                    qT_sb[hp:hp + 64,
                          c * S + sb * 512: c * S + (sb + 1) * 512],
                    tile_position=(hp, 0))
            nc.scalar.activation(exp_ab[half][:, tci * S:(tci + 1) * S],
                                 st[:], AF.Exp, scale=SCALE)

        def ctx_unit(cps, half, tci, c, exp_ab):
            h = 2 * c + half
            lhsT = v_sb[:, tci * VW + h * 65: tci * VW + h * 65 + 65]
            for sb in range(2):
                nc.tensor.matmul(
                    cps[:, sb * 512:(sb + 1) * 512], lhsT,
                    exp_ab[half][:, tci * S + sb * 512:
                                 tci * S + (sb + 1) * 512],
                    start=(tci == 0), stop=(tci == NC_S - 1))

        def prefix_scores(c, pexp):
            psp = st_pool.tile([128, S], F32, tag="st", name=f"psp_{c}")
            for half in range(2):
                hp = half * 64
                for sb in range(2):
                    nc.tensor.matmul(
                        psp[hp:hp + 64, sb * 512:(sb + 1) * 512],
                        pkT_sb[hp:hp + 64, c * AT:(c + 1) * AT],
                        qT_sb[hp:hp + 64,
                              c * S + sb * 512: c * S + (sb + 1) * 512],
                        tile_position=(hp, hp))
            nc.scalar.activation(pexp[:], psp[:], AF.Exp, scale=SCALE)

        def prefix_ctx(c, half, pexp):
            h = 2 * c + half
            hp = half * 64
            pps = ctx_pool.tile([65, S], F32, tag="ctx",
                                name=f"pps_{c}_{half}")
            for sb in range(2):
                nc.tensor.matmul(
                    pps[:, sb * 512:(sb + 1) * 512],
                    pv_sb[hp:hp + 64, h * 65: h * 65 + 65],
                    pexp[hp:hp + 64, sb * 512:(sb + 1) * 512],
                    tile_position=(hp, 0))
            return pps

        def finish_stage1(c, half, cps, pps):
            """Evacuate ctx+prefix numerators/denominators to SBUF, then
            launch the reciprocal-broadcast round trip (no PE involved)."""
            h = 2 * c + half
            nt = norm_pool.tile([65, 2 * S], F32, tag="nt", bufs=3,
                                name=f"nt_{c}_{half}")
            nc.vector.tensor_copy(nt[:, 0:S], cps[:])
            nc.vector.tensor_copy(nt[:, S:2 * S], pps[:])
            dresh = norm_pool.tile([128, 16], F32, tag="dresh", bufs=2,
                                   name=f"dr_{c}_{half}")
            nc.sync.dma_start(dresh[:, 0:8], nt[64:65, 0:S])
            nc.sync.dma_start(dresh[:, 8:16], nt[64:65, S:2 * S])
            rrec = norm_pool.tile([128, 16], BF16, tag="rrec", bufs=2,
                                  name=f"rr_{c}_{half}")
            with nc.allow_low_precision(reason="bf16 softmax recip bcast"):
                nc.vector.reciprocal(rrec[:], dresh[:])
            r_d = dscr_pool.tile([1, 2 * S], BF16, tag="rd", bufs=2,
                                 name=f"rd_{c}_{half}")
            nc.sync.dma_start(r_d[0:1, 0:S], rrec[:, 0:8])
            nc.sync.dma_start(r_d[0:1, S:2 * S], rrec[:, 8:16])
            r_bc = norm_pool.tile([64, 2 * S], BF16, tag="rbc", bufs=2,
                                  name=f"rbc_{c}_{half}")
            r_src = bass.AP(r_d[:].tensor, r_d[:].offset,
                            [[0, 64], [1, 2 * S]])
            nc.sync.dma_start(r_bc[:], r_src)
            return (h, nt, r_bc)

        def finish_stage2(state):
            """Normalize in place, combine, write out (all DVE)."""
            h, nt, r_bc = state
            nc.vector.tensor_mul(nt[0:64, 0:S], nt[0:64, 0:S], r_bc[:, 0:S])
            nc.vector.tensor_mul(nt[0:64, S:2 * S], nt[0:64, S:2 * S],
                                 r_bc[:, S:2 * S])
            nc.vector.tensor_add(nt[0:64, 0:S], nt[0:64, 0:S],
                                 nt[0:64, S:2 * S])
            nc.sync.dma_start(outT[h * 64:(h + 1) * 64, :], nt[0:64, 0:S])

        # ================= the window schedule =================
        # HAM warm-up: ~4us of tiny full-array matmuls on scratch data
        # while the input DMAs stream, so the PE clock is at 2.4GHz when
        # the first projection matmul issues.
        nc.vector.memset(warm_sb[:], 0.0)
        wps = st_pool.tile([128, S], F32, tag="st", name="warmps")
        for i in range(50):
            nc.tensor.matmul(wps[:, 0:128], warm_sb[:], warm_sb[:, 0:128])

        # chunk-0 Q/K projections up front (pair 0 inputs); everything
        # else rides as window fillers behind the exp stream
        qk_half(0, 0)
        qk_half(0, 1)

        # filler units per window (emitted between score units).
        # window 0 also carries the prompt-K projections so pair-0's
        # prefix branch can run at the uniform half-lag schedule.
        fillers = [[] for _ in range(PAIRS)]
        fillers[0] = [lambda: qk_half(1, 0), lambda: qk_half(1, 1)] + [
            lambda sc=sc: v_chunk(sc) for sc in range(5)] + [
            lambda cc=cc: prompt_k(cc) for cc in range(NC_D)]
        fillers[1] = [prompt_v, lambda: qk_half(2, 0), lambda: qk_half(2, 1),
                      lambda: qk_half(3, 0), lambda: qk_half(3, 1)]
        fillers[2] = [lambda: qk_half(4, 0), lambda: qk_half(4, 1)]
        fillers[3] = [lambda: qk_half(5, 0), lambda: qk_half(5, 1)]
        # v s5-7 must land before pair-0's leftover ctx units read them:
        # v5/v6 ahead of the tci-0 leftovers, v7 ahead of the tci-1 ones
        front = [{} for _ in range(PAIRS)]
        front[1] = {0: [lambda: v_chunk(5), lambda: v_chunk(6)],
                    1: [lambda: v_chunk(7)]}

        # Half-lag schedule per window c:
        #   tci 0-1: leftover ctx units (ti 5,6,7) of pair c-1
        #   tci 2:   prefix-ctx + finish stage1 of pair c-1
        #   tci 3-7: ctx units ti 0..4 of pair c (1 ti per tci)
        #   tci 6:   finish stage2 of pair c-1 (r_bc round trip done)
        #   end:     prefix scores of pair c (joins the exp stream)
        prev = None      # (c, exp_ab, pexp, cps_ab, leftover list)
        stage2_q = []

        for c in range(PAIRS):
            exp_ab = [exp_pool.tile([128, NC_S * S], BF16, tag="exp",
                                    name=f"exp_{c}_{i}") for i in range(2)]
            fl = list(fillers[c])
            nfill = len(fl)
            cps_ab = None
            states = []
            # window 5 runs its whole ctx in-window (no next window)
            ctx_plan = [(half, ti) for ti in range(NC_S)
                        for half in range(2)]
            quota = 16 if c == PAIRS - 1 else 10
            for tci in range(NC_S):
                for half in range(2):
                    score_unit(c, tci, half, exp_ab)
                for fn in front[c].get(tci, ()):
                    fn()
                if c == 0 and tci == 1:
                    gating_setup()
                if prev is not None:
                    pc, pexp_ab, ppexp, pcps, pleft = prev
                    if tci < 2:
                        for _ in range(3):
                            if pleft:
                                half, ti = pleft.pop(0)
                                ctx_unit(pcps[half], half, ti, pc, pexp_ab)
                    elif tci == 2:
                        for half in range(2):
                            pps = prefix_ctx(pc, half, ppexp)
                            states.append((half, pps))
                        for half, pps in states:
                            stage2_q.append(
                                finish_stage1(pc, half, pcps[half], pps))
                if tci >= 3:
                    if cps_ab is None:
                        cps_ab = {hh: ctx_pool.tile(
                            [65, S], F32, tag="ctx", name=f"cps_{c}_{hh}")
                            for hh in range(2)}
                    done = 16 - len(ctx_plan)
                    want = quota * (tci - 2) // 5 - done
                    for _ in range(max(0, want)):
                        if ctx_plan:
                            half, ti = ctx_plan.pop(0)
                            ctx_unit(cps_ab[half], half, ti, c, exp_ab)
                if tci == 6:
                    for stt in stage2_q:
                        finish_stage2(stt)
                    stage2_q = []
                while fl and len(fl) > nfill * (7 - tci) // 8:
                    fl.pop(0)()

            pexp = pexp_pool.tile([128, S], BF16, tag="pexp",
                                  name=f"pexp_{c}")
            prefix_scores(c, pexp)
            prev = (c, exp_ab, pexp, cps_ab, ctx_plan)

        # ---- tail: pair 5 remaining ctx + stage-parallel finish ----
        pc, pexp_ab, ppexp, pcps, pleft = prev
        for half, ti in pleft:
            ctx_unit(pcps[half], half, ti, pc, pexp_ab)
        for stt in stage2_q:
            finish_stage2(stt)
        pps_l = [prefix_ctx(pc, half, ppexp) for half in range(2)]
        nts, rrecs, rps = [], [], []
        for half in range(2):
            nt = norm_pool.tile([65, 2 * S], F32, tag="nt", bufs=3,
                                name=f"ntt_{half}")
            nc.vector.tensor_copy(nt[:, 0:S], pcps[half][:])
            nc.vector.tensor_copy(nt[:, S:2 * S], pps_l[half][:])
            nts.append(nt)
        for half in range(2):
            dresh = norm_pool.tile([128, 16], F32, tag="dresh", bufs=2,
                                   name=f"drt_{half}")
            nc.sync.dma_start(dresh[:, 0:8], nts[half][64:65, 0:S])
            nc.sync.dma_start(dresh[:, 8:16], nts[half][64:65, S:2 * S])
            rrec = norm_pool.tile([128, 16], BF16, tag="rrec", bufs=2,
                                  name=f"rrt_{half}")
            with nc.allow_low_precision(reason="softmax recip bcast"):
                nc.vector.reciprocal(rrec[:], dresh[:])
            rrecs.append(rrec)
        for half in range(2):
            r_row = norm_pool.tile([1, 2 * S], BF16, tag="rrow", bufs=2,
                                   name=f"rwt_{half}")
            nc.sync.dma_start(r_row[0:1, 0:S], rrecs[half][:, 0:8])
            nc.sync.dma_start(r_row[0:1, S:2 * S], rrecs[half][:, 8:16])
            rps_c = st_pool.tile([128, S], F32, tag="st",
                                 name=f"rpc_{half}")
            rps_p = ctx_pool.tile([65, S], F32, tag="ctx",
                                  name=f"rpp_{half}")
            for sb in range(2):
                nc.tensor.matmul(rps_c[0:64, sb * 512:(sb + 1) * 512],
                                 ones_sb[:],
                                 r_row[0:1, sb * 512:(sb + 1) * 512])
                nc.tensor.matmul(rps_p[0:64, sb * 512:(sb + 1) * 512],
                                 ones_sb[:],
                                 r_row[0:1, S + sb * 512: S + (sb + 1) * 512])
            rps.append((rps_c, rps_p))
        for half in range(2):
            h = 2 * pc + half
            nt = nts[half]
            rps_c, rps_p = rps[half]
            nc.vector.tensor_mul(nt[0:64, 0:S], nt[0:64, 0:S],
                                 rps_c[0:64, :])
            nc.vector.tensor_mul(nt[0:64, S:2 * S], nt[0:64, S:2 * S],
                                 rps_p[0:64, :])
            nc.gpsimd.tensor_add(nt[0:64, 0:S], nt[0:64, 0:S],
                                 nt[0:64, S:2 * S])
            nc.sync.dma_start(outT[h * 64:(h + 1) * 64, :], nt[0:64, 0:S])


def _prep_inputs(hidden_states, prompt_tokens, gating_factor, attention_mask,
                 Wq, bq, Wk, bk, Wv, bv):
    bf = ml_dtypes.bfloat16
    hs = np.asarray(hidden_states, np.float32)
    mask = np.asarray(attention_mask, np.float32).reshape(B, S)
    def _cmaj(w):
        # [din, dout] -> [128, (c, kc, j)] so each chunk DMA is a plain
        # contiguous column-slice (1.5KB per partition line)
        wt = np.asarray(w, np.float32).T.reshape(NC_D, 128, NC_D, 128)
        return np.ascontiguousarray(
            wt.transpose(1, 2, 0, 3).reshape(128, NC_D * D)).astype(bf)

    wqT = _cmaj(Wq)
    wkT = _cmaj(Wk)
    # augmented WvT: [din, 780], col 65h+j = Wv.T[:, 64h+j], col 65h+64 = 0
    wvT_f = np.asarray(Wv, np.float32).T  # [din, dout]
    wvT_aug = np.zeros((D, VW), np.float32)
    idx = np.arange(D)
    aug_cols = (idx // DH) * (DH + 1) + (idx % DH)
    wvT_aug[:, aug_cols] = wvT_f
    wvT_aug = wvT_aug.astype(bf)
    bq_c = np.asarray(bq, np.float32).reshape(D, 1)
    bk_c = np.asarray(bk, np.float32).reshape(D, 1)
    bv_aug = np.zeros(VW, np.float32)
    bv_aug[aug_cols] = np.asarray(bv, np.float32)
    bv_aug[DH::DH + 1] = 1.0
    bvaug_bc = np.ascontiguousarray(
        np.broadcast_to(bv_aug, (128, VW)), np.float32)
    p0T = np.asarray(prompt_tokens, np.float32)[0].T.reshape(NC_D, 128, AT)
    pT = np.ascontiguousarray(
        p0T.transpose(1, 0, 2).reshape(128, NC_D * AT)).astype(bf)
    gat_row = np.repeat(
        np.asarray(gating_factor, np.float32).reshape(H), DH + 1)
    gat = np.ascontiguousarray(
        np.broadcast_to(gat_row, (128, VW)), np.float32)

    shared = dict(wqT=wqT, wkT=wkT, wvT=wvT_aug, bq=bq_c, bk=bk_c,
                  bvaug=bvaug_bc, promptT=pT, gating=gat)
    in_maps = []
    for b in range(B):
        m = dict(shared)
        m["hsT"] = np.ascontiguousarray(hs[b].T).astype(bf)
        m["mask"] = np.ascontiguousarray(mask[b].reshape(S, 1))
        in_maps.append(m)
    return in_maps


def kernel(**inputs):
    global LAST_RESULTS
    if "nc" not in _CACHE:
        _CACHE["nc"] = _build_nc()
    nc = _CACHE["nc"]
    in_maps = _prep_inputs(**inputs)
    res = None
    for attempt in range(3):
        try:
            res = run_bass_kernel_spmd(nc, in_maps, list(range(B)))
            break
        except ModuleNotFoundError:
            # BASS_TRACE set but this image lacks antenv.axon_hooks
            import os

            os.environ["BASS_NEVER_TRACE"] = "1"
            if attempt == 2:
                raise
        except Exception:
            # transient NRT_EXEC_UNIT_UNRECOVERABLE on a cold device has
            # been observed; a retry on the same session recovers
            if attempt == 2:
                raise
    LAST_RESULTS = res
    out = np.empty((B, S, D), np.float32)
    for b in range(B):
        out[b] = res.results[b]["outT"].T
    return out
